# revision 48
# baseline (speedup 1.0000x reference)
"""GridAttention Trainium2 kernel (v5: ramp/ring/interleave tuning).

Full inputs -> full output. Internally shards (batch, head-pair) across 8
NeuronCores: core c handles batch c//4 and heads (2*(c%4), 2*(c%4)+1).

Math notes:
 - Attention scores are computed TRANSPOSED: S^T[j, i] = k_j . q_i * scale
   + rowbias[i, j], laid out [k partitions, q free]. Softmax-exp is
   elementwise, the denominator a matmul reduction (ones column in V), and
   P^T is directly the moving operand of the AV matmul.
 - ROW bias rides inside the QK matmul (contraction augmented to
   K=128 = [qk 64 | onehot(rj) 64] against [q 64 | rowr 64]).
 - COL bias applied multiplicatively after exp (9 distinct periodic
   pair-blocks, host-precomputed as ecol).
 - No max-subtraction in softmax (logits ~ N(0,1), shift-invariant).
 - Device emits per-head UNNORMALIZED projected output + denominator
   row; host computes sum_h out_h / d_h. (Device-side normalization
   was tried: its DVE recip/broadcast chain costs more than it saves,
   and the custom-DVE fast reciprocal mis-reads PSUM at nonzero base
   partition on HW.)

Schedule notes (trace-driven):
 - Engine floors per core: scalar exp 144 x ~1.08us = ~156us is the
   pacer; PE ~147us; DVE ~150us. The kernel is a race to start the exp
   stream early (~14us) and never stall it.
 - Input DMAs split across the three DMA queues (sync/SP HWDGE,
   scalar/Act HWDGE, gpsimd SWDGE), ~64-72GB/s each, parallel. wk/wq
   lead sync/scalar so the first k/q projections start ~10us; xT
   chunks are halved across rings against their scores deadlines.
 - onehot(row) [64, S] generated on device from the identity via a
   stride-0 broadcast copy (DVE), saving 786KB of ramp DMA.
 - First four groups interleave heads so early scores need only
   k-chunk 0 (+2 groups of slack on later xT deadlines).
 - Scalar engine does exp ONLY between first and last exp. GpSimd does
   NO tensor work mid-loop (a v2 experiment showed gpsimd tensor_mul
   contends for SBUF and slows concurrent DVE ops ~50%): it only
   issues SWDGE DMAs. All multiplies/evacuations/casts on DVE.
 - vv ones+values combined: [v_h0 64 | one | v_h1 64 | one] per
   130-col block: each v-transpose needs ONE (2-block-AP) copy and
   both heads' AV lhsT slices are partition-aligned [v|one].
"""

import numpy as np

EMBED = 512
NH = 8
HD = 64
GH, GW = 64, 48
B = 2
S = GH * GW  # 3072
N_CORES = 8
NQ = S // 512  # 6 q chunks of 512
NM = S // 128  # 24 k chunks of 128
NG = NM // 2   # 12 groups of 2 k-chunks per (n, h)
KC = 4         # 512 = 4 contraction chunks of 128

_CACHE = {}


def _build_program():
    import concourse.bass as bass
    import concourse.tile as tile
    import concourse.mybir as mybir
    from concourse import bacc
    from concourse.bass import ts, ds
    from concourse.masks import make_identity

    f32 = mybir.dt.float32
    f16 = mybir.dt.float16
    EXP = mybir.ActivationFunctionType.Exp

    nc = bacc.Bacc("TRN2", target_bir_lowering=False, debug=False,
                   num_devices=N_CORES)

    def inp(name, shape):
        return nc.dram_tensor(name, shape, f16, kind="ExternalInput").ap()

    # host-prepacked layouts (see _prep_core_inputs)
    xT_d = inp("xT", [128, NQ * 2048])        # [p, n*2048 + c*512 + col]
    wqkv_d = inp("wqkv", [128, 3 * 512])      # [p, (q|k|v)*512 + c*128 + col]
    rowr_d = [inp(f"rowr{h}", [64, S]) for h in range(2)]      # rowr_h only
    ecol_d = [inp(f"ecol{h}", [128, 6144]) for h in range(2)]  # blocks 0,1,2,0
    wout_d = inp("wout", [HD, 2 * EMBED])
    outa_d = nc.dram_tensor("outa", [S, EMBED], f16, kind="ExternalOutput").ap()
    outb_d = nc.dram_tensor("outb", [S, EMBED], f16, kind="ExternalOutput").ap()
    den_d = nc.dram_tensor("den", [2, S], f16, kind="ExternalOutput").ap()

    with tile.TileContext(nc) as tc:
        with (
            tc.tile_pool(name="const", bufs=1) as cpool,
            tc.tile_pool(name="vtwp", bufs=3) as vtwp,
            tc.tile_pool(name="ptp", bufs=4) as ptp,
            tc.tile_pool(name="ptmp", bufs=6) as ptmp,
            tc.tile_pool(name="osb", bufs=3) as opool,
            tc.tile_pool(name="ps", bufs=2, space="PSUM") as ps,
        ):
            # ---- resident SBUF tensors ----
            xT = [cpool.tile([128, 2048], f16, tag=f"xT{n}", name=f"xT{n}")
                  for n in range(NQ)]
            wqkv = cpool.tile([128, 3 * 512], f16)
            wout = cpool.tile([HD, 2 * EMBED], f16)
            augLR = [cpool.tile([128, 2 * S], f16, tag=f"augLR{h}",
                                name=f"augLR{h}") for h in range(2)]
            ecol = [cpool.tile([128, 6144], f16, tag=f"ecol{h}",
                               name=f"ecol{h}") for h in range(2)]
            # per 130-block: [v_h0 64 | one | v_h1 64 | one] so each
            # head's AV lhsT slice is a partition-0-aligned [v|one]
            vv = cpool.tile([128, NM * 130], f16)
            outT = [cpool.tile([65, S], f16, tag=f"outT{h}", name=f"outT{h}")
                    for h in range(2)]
            ident = cpool.tile([128, 128], f16)

            # ---- input DMA rings ------------------------------------
            # HWDGE ring depth is 4: the 5th+ dma_start on a queue
            # BLOCKS the queue until a transfer slot frees. The scalar
            # queue therefore gets EXACTLY 4 pre-exp items (its queue
            # must be free when exp(0) is ready ~16us); sync and gpsimd
            # queues have nothing time-critical and absorb the rest.
            nc.gpsimd.dma_start(out=wqkv[:, 1024:1536],
                                in_=wqkv_d[:, 1024:1536])    # wv
            make_identity(nc, ident[:, :])
            nc.gpsimd.memset(vv[:, 0:512], 1.0)
            nc.gpsimd.memset(vv[:, 512:NM * 130], 1.0)
            nc.gpsimd.dma_start(out=ecol[1][:, 0:1024],
                                in_=ecol_d[1][:, 0:1024])
            nc.gpsimd.dma_start(out=ecol[1][:, 1024:2048],
                                in_=ecol_d[1][:, 1024:2048])
            nc.gpsimd.dma_start(out=xT[1][:, 1024:2048],
                                in_=xT_d[:, ds(1 * 2048 + 1024, 1024)])
            nc.gpsimd.dma_start(out=xT[2][:, 0:1024],
                                in_=xT_d[:, ds(2 * 2048, 1024)])
            nc.gpsimd.dma_start(out=xT[2][:, 1024:2048],
                                in_=xT_d[:, ds(2 * 2048 + 1024, 1024)])
            nc.gpsimd.dma_start(out=xT[3][:, 1024:2048],
                                in_=xT_d[:, ds(3 * 2048 + 1024, 1024)])
            nc.gpsimd.dma_start(out=xT[4][:, 0:1024],
                                in_=xT_d[:, ds(4 * 2048, 1024)])
            nc.gpsimd.dma_start(out=xT[5][:, 1024:2048],
                                in_=xT_d[:, ds(5 * 2048 + 1024, 1024)])

            # sync (SP HWDGE): wk first (gates kproj0), xT0 c0/c2,
            # rowr n0 slices, xT1a, ecol0 panel-0, late a-halves
            nc.sync.dma_start(out=wqkv[:, 512:1024], in_=wqkv_d[:, 512:1024])
            nc.sync.dma_start(out=xT[0][:, 0:512], in_=xT_d[:, ds(0, 512)])
            nc.sync.dma_start(out=xT[0][:, 1024:1536],
                              in_=xT_d[:, ds(1024, 512)])
            nc.sync.dma_start(out=augLR[0][64:128, ds(S, 512)],
                              in_=rowr_d[0][:, 0:512])
            nc.sync.dma_start(out=augLR[1][64:128, ds(S, 512)],
                              in_=rowr_d[1][:, 0:512])
            nc.sync.dma_start(out=xT[1][:, 0:1024],
                              in_=xT_d[:, ds(1 * 2048, 1024)])
            nc.sync.dma_start(out=ecol[0][:, 0:1024],
                              in_=ecol_d[0][:, 0:1024])
            nc.sync.dma_start(out=xT[3][:, 0:1024],
                              in_=xT_d[:, ds(3 * 2048, 1024)])
            nc.sync.dma_start(out=xT[5][:, 0:1024],
                              in_=xT_d[:, ds(5 * 2048, 1024)])

            # scalar (Act HWDGE): 5 items — the 5th issue ring-blocks
            # the queue only until the 1st transfer completes (~12us),
            # still well before exp(0)
            nc.scalar.dma_start(out=wqkv[:, 0:512], in_=wqkv_d[:, 0:512])
            nc.scalar.dma_start(out=xT[0][:, 512:1024],
                                in_=xT_d[:, ds(512, 512)])
            nc.scalar.dma_start(out=xT[0][:, 1536:2048],
                                in_=xT_d[:, ds(1536, 512)])
            nc.scalar.dma_start(out=xT[4][:, 1024:2048],
                                in_=xT_d[:, ds(4 * 2048 + 1024, 1024)])
            nc.scalar.dma_start(out=ecol[0][:, 1024:2048],
                                in_=ecol_d[0][:, 1024:2048])

            # ---- onehot(row) [64, S] generated on device ------------
            # ohr = kron(I64, ones(1,48)): identity columns repeated 48x
            # via a stride-0 inner AP; split at col 1056 (=48*22)
            def emit_ohr(h, c0, c1, eng):
                src = ident[0:64, c0:c1]
                src = bass.AP(src.tensor, src.offset, src.ap + [[0, 48]])
                dst = augLR[h][64:128, 48 * c0:48 * c1]
                dst = bass.AP(dst.tensor, dst.offset,
                              [dst.ap[0], [48, c1 - c0], [1, 48]])
                if hasattr(eng, "tensor_copy"):
                    eng.tensor_copy(dst, src)
                else:
                    eng.copy(dst, src)

            # ---- group order: heads fully interleaved --------------
            # (n,0,g),(n,1,g) pairs: each k-chunk pair is consumed over
            # TWO groups, halving the k-side DMA bandwidth demand during
            # the ramp (xT_n deadline ~G(4n) instead of ~G(2n)).
            groups = [(n, h, g) for n in range(NQ) for g in range(NG)
                      for h in (0, 1)]
            NGRP = len(groups)
            assert NGRP == NQ * 2 * NG

            live = {}
            acc = {}

            def emit_scores(i):
                n, h, g = groups[i]
                st = ps.tile([128, 1024], f32, tag="st", name="st")
                for k in range(2):
                    m = 2 * g + k
                    nc.tensor.matmul(st[:, ts(k, 512)],
                                     augLR[h][:, ts(m, 128)],
                                     augLR[h][:, ds(S + n * 512, 512)],
                                     start=True, stop=True)
                live[("st", i)] = st

            def emit_expmul(i):
                n, h, g = groups[i]
                st = live.pop(("st", i))
                pt = ptp.tile([128, 1024], f16, tag="pt", name="pt")
                nc.scalar.activation(pt[:, :], st[:, :], EXP)
                ptm = ptmp.tile([128, 1024], f16, tag="ptm", name="ptm")
                esl = ecol[h][:, ds((n % 3) * 2048 + (2 * g % 3) * 512,
                                    1024)]
                # NOTE: scalar_tensor_tensor looks 4x-capable in the
                # cost model but measures ~2.5x SLOWER than tensor_mul
                # on hardware; plain tensor_tensor 2x (~640ns) is best.
                nc.vector.tensor_mul(ptm[:, :], pt[:, :], esl)
                live[("ptm", i)] = ptm

            def emit_av(i):
                n, h, g = groups[i]
                ptm = live.pop(("ptm", i))
                if g == 0:
                    acc[(n, h)] = ps.tile([65, 512], f32, tag="acc",
                                          name="acc")
                a = acc[(n, h)]
                for k in range(2):
                    m = 2 * g + k
                    nc.tensor.matmul(a[:, :], vv[:, ds(m * 130 + 65 * h, 65)],
                                     ptm[:, ts(k, 512)],
                                     start=(m == 0), stop=(m == NM - 1))
                if g == NG - 1:
                    # acc rows = [v64, den]: emit the unnormalized head
                    # output + den row; host divides and combines
                    # (device-side normalization was tried: its DVE
                    # recip/broadcast chain costs more than it saves).
                    # Last chunk: split halves so the final tail units
                    # pipeline with the copy.
                    if n == NQ - 1:
                        nc.vector.tensor_copy(
                            outT[h][:, ds(n * 512, 256)], a[:, 0:256])
                        nc.vector.tensor_copy(
                            outT[h][:, ds(n * 512 + 256, 256)], a[:, 256:512])
                    else:
                        nc.vector.tensor_copy(outT[h][:, ts(n, 512)],
                                              a[:, :])
                    del acc[(n, h)]

            def emit_tail_unit(t, h, tag="fp", cast_scalar=False,
                               ring="default"):
                fp = ps.tile([128, 512], f32, tag=tag, name="fp")
                nc.tensor.matmul(fp[:, :], outT[h][0:64, ts(t, 128)],
                                 wout[:, ds(h * EMBED, EMBED)],
                                 start=True, stop=True)
                osb = opool.tile([128, 512], f16, tag="osb", name="osb")
                if cast_scalar:
                    nc.scalar.copy(osb[:, :], fp[:, :])
                else:
                    nc.vector.tensor_copy(osb[:, :], fp[:, :])
                if ring == "default":
                    eng = nc.sync if h == 0 else nc.gpsimd
                else:
                    eng = ring
                out_d = outa_d if h == 0 else outb_d
                eng.dma_start(out=out_d[ts(t, 128), :], in_=osb[:, :])

            def proj(dst_tag, w_ofs, n, tag):
                p = ps.tile([128, 512], f32, tag=tag, name=f"p{dst_tag}")
                for c in range(KC):
                    nc.tensor.matmul(p[:, :], wqkv[:, ds(w_ofs + c * 128, 128)],
                                     xT[n][:, ts(c, 512)],
                                     start=(c == 0), stop=(c == KC - 1))
                return p

            def emit_kevac(n, pk, h_first_only=False):
                nc.vector.tensor_copy(augLR[0][0:64, ts(n, 512)], pk[0:64, :])
                if not h_first_only:
                    nc.vector.tensor_copy(augLR[1][0:64, ts(n, 512)],
                                          pk[64:128, :])

            def emit_qproj_copy(pq, n, h_list=(0, 1)):
                for h in h_list:
                    nc.vector.tensor_copy(augLR[h][0:64, ds(S + n * 512, 512)],
                                          pq[64 * h:64 * h + 64, :])

            def emit_vproj(n):
                pv = proj("v", 1024, n, "fp")
                vtw = vtwp.tile([128, 512], f16, tag="vtw", name="vtw")
                nc.vector.tensor_copy(vtw[:, :], pv[:, :])
                for mm in range(4):
                    m = n * 4 + mm
                    ptr = ps.tile([128, 128], f16, tag="fp", name="ptr")
                    nc.tensor.transpose(ptr[:, :], vtw[:, ts(mm, 128)],
                                        ident[:, :])
                    # one copy into both heads' v slots, skipping the
                    # ones column at block offset 64 via a 2-block AP
                    s = ptr[:, :]
                    s = bass.AP(s.tensor, s.offset,
                                [s.ap[0], [64, 2], [1, 64]])
                    d = vv[:, ds(m * 130, 129)]
                    d = bass.AP(d.tensor, d.offset,
                                [d.ap[0], [65, 2], [1, 64]])
                    nc.vector.tensor_copy(d, s)

            # ---- ramp ----------------------------------------------
            warm = ps.tile([128, 512], f32, tag="st", name="warm")

            def emit_warm(k):
                for _ in range(k):
                    nc.tensor.matmul(warm[:, :], ident[:, :], vv[:, 0:512],
                                     start=True, stop=True)

            emit_ohr(0, 0, 22, nc.vector)
            emit_ohr(1, 0, 22, nc.vector)
            # h0 cols 1056:3072 on the scalar queue (idle until
            # exp(0)); h1 piece stays on DVE at loop i==4 — both on
            # scalar would hold the queue past exp(0)'s ready time
            emit_ohr(0, 22, 64, nc.scalar)

            emit_warm(6)
            pk0 = proj("k", 512, 0, "fp")
            emit_warm(2)
            pq0 = proj("q", 0, 0, "fp")
            # h0 evacs first: scores(0) only needs head 0
            emit_kevac(0, pk0, h_first_only=True)
            emit_qproj_copy(pq0, 0, h_list=(0,))
            emit_scores(0)   # (0,0,0)
            nc.vector.tensor_copy(augLR[1][0:64, ts(0, 512)], pk0[64:128, :])
            emit_qproj_copy(pq0, 0, h_list=(1,))
            emit_expmul(0)
            emit_vproj(0)    # vv chunks 0..3; AV(G0) lands at loop i=3
            emit_warm(3)
            emit_scores(1)   # (0,1,0)
            emit_expmul(1)
            pk1 = proj("k", 512, 1, "fp")
            emit_kevac(1, pk1)

            # ---- main loop (AV lagged 3 groups behind scores/exp) ----
            pq = {}
            for i in range(2, NGRP):
                n, h, g = groups[i]
                if i == 4:
                    emit_ohr(1, 22, 64, nc.vector)
                emit_scores(i)
                emit_expmul(i)
                if i >= 3:
                    emit_av(i - 3)
                # kproj/vproj alternate slots (together in one slot
                # they exhaust the PE 4-deep wait queue and head-block
                # the next scores)
                if i in (4, 8, 12, 16, 20):
                    emit_vproj(i // 4)
                # kproj nk consumed by scores (0,*,2(nk-1)) = G(4nk-4)
                if i in (2, 6, 10, 14):
                    nk = (i + 6) // 4
                    pkn = proj("k", 512, nk, "fp")
                    emit_kevac(nk, pkn)
                # lazy-constant DMA issues from gpsimd queue slots
                if i == 3:   # rowr0 n1..n5
                    nc.gpsimd.dma_start(out=augLR[0][64:128, ds(S + 512, 2560)],
                                        in_=rowr_d[0][:, ds(512, 2560)])
                if i == 5:   # rowr1 n1..n5
                    nc.gpsimd.dma_start(out=augLR[1][64:128, ds(S + 512, 2560)],
                                        in_=rowr_d[1][:, ds(512, 2560)])
                if i == 7:   # wout (first tail unit ~G26)
                    nc.gpsimd.dma_start(out=wout[:, :], in_=wout_d[:, :])
                # ecol p1/p2 panels ride the sync ring (idle after its
                # ramp items; the gpsimd ring is backed up with xT)
                if i == 9:   # ecol0 panel 1 ((1,0) muls ~G24)
                    nc.sync.dma_start(out=ecol[0][:, 2048:4096],
                                      in_=ecol_d[0][:, 2048:4096])
                if i == 15:  # ecol1 panel 1
                    nc.sync.dma_start(out=ecol[1][:, 2048:4096],
                                      in_=ecol_d[1][:, 2048:4096])
                if i == 21:  # ecol0 panel 2
                    nc.sync.dma_start(out=ecol[0][:, 4096:6144],
                                      in_=ecol_d[0][:, 4096:6144])
                if i == 27:  # ecol1 panel 2
                    nc.sync.dma_start(out=ecol[1][:, 4096:6144],
                                      in_=ecol_d[1][:, 4096:6144])
                # q-proj n=1 during the (0,1) block
                if (n, h, g) == (0, 1, 3):
                    pq[1] = proj("q", 0, 1, "fp")
                if (n, h, g) == (0, 1, 5):
                    emit_qproj_copy(pq.pop(1), 1)
                # output-projection tail units: outT[*] chunk n-1 is
                # complete by loop i=24n+1; slots g=4..7 of both heads
                # in block n emit its 8 units (g>=4 keeps the unit
                # casts clear of the block-boundary DVE burst of outT
                # copies, which was stalling the muls ~1.7us per block)
                if n >= 1 and 4 <= g < 8:
                    emit_tail_unit(4 * (n - 1) + (g - 4), h)
                if h == 0 and g == 6 and 2 <= n + 2 < NQ:
                    pq[n + 2] = proj("q", 0, n + 2, "fp")
                if h == 0 and g == 8 and 2 <= n + 2 < NQ:
                    emit_qproj_copy(pq.pop(n + 2), n + 2)
            for i in range(NGRP - 3, NGRP):
                emit_av(i)
            # denominator rows: den0 on sync (outT[0] done well before
            # the end), den1 on scalar (free after the last exp)
            nc.sync.dma_start(out=den_d[0:1, :], in_=outT[0][64:65, :])
            nc.scalar.dma_start(out=den_d[1:2, :], in_=outT[1][64:65, :])
            # final-chunk tail: 8 units, casts split DVE/scalar, DMAs
            # spread over three rings so the drain doesn't serialize
            tail_rings = [nc.sync, nc.gpsimd, nc.scalar, nc.sync,
                          nc.gpsimd, nc.scalar, nc.sync, nc.gpsimd]
            for tt in range(4):
                for hh in range(2):
                    k = 2 * tt + hh
                    emit_tail_unit(4 * (NQ - 1) + tt, hh,
                                   tag="fp" if k % 2 == 0 else "st",
                                   cast_scalar=(k % 2 == 1),
                                   ring=tail_rings[k])

    nc.compile()
    return nc


def _get_nc():
    if "nc" not in _CACHE:
        _CACHE["nc"] = _build_program()
    return _CACHE["nc"]


def _prep_core_inputs(x, w_qkv, w_out, rel_row_tab, rel_col_tab):
    """Per-core input dicts (host-side shard + constant precompute)."""
    bf = np.float16
    x = np.asarray(x, np.float32)
    w_qkv = np.asarray(w_qkv, np.float32)
    w_out = np.asarray(w_out, np.float32)
    rel_row_tab = np.asarray(rel_row_tab, np.float32)
    rel_col_tab = np.asarray(rel_col_tab, np.float32)

    ri = np.arange(S) // GW
    row_idx = ri[None, :] - np.arange(64)[:, None] + 63   # [64, S]

    jj = np.arange(128)
    ii = np.arange(512)
    ecol_idx = np.zeros((3, 4, 128, 512), np.int64)
    for q3 in range(3):
        for bp in range(4):
            cj = (32 * (bp % 3) + jj) % 48
            c_i = (32 * q3 + ii) % 48
            ecol_idx[q3, bp] = c_i[None, :] - cj[:, None] + 47
    ecol_idx = ecol_idx.transpose(2, 0, 1, 3).reshape(128, 6144)

    scale = HD ** -0.5
    in_maps = []
    for c in range(N_CORES):
        b = c // 4
        h0 = 2 * (c % 4)
        h1 = h0 + 1
        xT = np.ascontiguousarray(x[b].reshape(S, EMBED).T)   # [E, S]
        xTn = xT.reshape(KC, 128, NQ, 512).transpose(1, 2, 0, 3)
        def wslice(base, h):
            return w_qkv[:, base + h * HD: base + (h + 1) * HD]
        def pack(base, mul=1.0):
            w = np.concatenate([wslice(base, h0), wslice(base, h1)],
                               axis=1) * mul                  # [512, 128]
            return w.reshape(KC, 128, 128).transpose(1, 0, 2).reshape(128, 512)
        wqkv = np.concatenate([pack(0, scale), pack(EMBED), pack(2 * EMBED)],
                              axis=1)

        in_maps.append({
            "xT": np.ascontiguousarray(xTn.reshape(128, NQ * 2048)).astype(bf),
            "wqkv": np.ascontiguousarray(wqkv).astype(bf),
            "rowr0": np.ascontiguousarray(
                rel_row_tab[row_idx, h0]).astype(bf),
            "rowr1": np.ascontiguousarray(
                rel_row_tab[row_idx, h1]).astype(bf),
            "ecol0": np.exp(rel_col_tab[ecol_idx, h0]).astype(bf),
            "ecol1": np.exp(rel_col_tab[ecol_idx, h1]).astype(bf),
            "wout": np.concatenate(
                [w_out[h0 * HD:(h0 + 1) * HD, :],
                 w_out[h1 * HD:(h1 + 1) * HD, :]], axis=1).astype(bf),
        })
    return in_maps


def _run(inputs, trace=False):
    from concourse.bass_utils import run_bass_kernel_spmd
    nc = _get_nc()
    in_maps = _prep_core_inputs(**inputs)
    res = run_bass_kernel_spmd(nc, in_maps, list(range(N_CORES)), trace=trace)
    acc = np.zeros((B, S, EMBED), np.float32)
    for c in range(N_CORES):
        r = res.results[c]
        den = np.asarray(r["den"], np.float32)          # [2, S]
        acc[c // 4] += np.asarray(r["outa"], np.float32) / den[0][:, None]
        acc[c // 4] += np.asarray(r["outb"], np.float32) / den[1][:, None]
    return acc.reshape(B, GH, GW, EMBED), res


def kernel(x, w_qkv, w_out, rel_row_tab, rel_col_tab):
    out, _ = _run(dict(x=x, w_qkv=w_qkv, w_out=w_out,
                       rel_row_tab=rel_row_tab, rel_col_tab=rel_col_tab))
    return out


# revision 54
# speedup vs baseline: 1.0119x; 1.0119x over previous
"""GridAttention Trainium2 kernel (v5: ramp/ring/interleave tuning).

Full inputs -> full output. Internally shards (batch, head-pair) across 8
NeuronCores: core c handles batch c//4 and heads (2*(c%4), 2*(c%4)+1).

Math notes:
 - Attention scores are computed TRANSPOSED: S^T[j, i] = k_j . q_i * scale
   + rowbias[i, j], laid out [k partitions, q free]. Softmax-exp is
   elementwise, the denominator a matmul reduction (ones column in V), and
   P^T is directly the moving operand of the AV matmul.
 - ROW bias rides inside the QK matmul (contraction augmented to
   K=128 = [qk 64 | onehot(rj) 64] against [q 64 | rowr 64]).
 - COL bias applied multiplicatively after exp (9 distinct periodic
   pair-blocks, host-precomputed as ecol).
 - No max-subtraction in softmax (logits ~ N(0,1), shift-invariant).
 - Device emits per-head UNNORMALIZED projected output + denominator
   row; host computes sum_h out_h / d_h. (Device-side normalization
   was tried: its DVE recip/broadcast chain costs more than it saves,
   and the custom-DVE fast reciprocal mis-reads PSUM at nonzero base
   partition on HW.)

Schedule notes (trace-driven):
 - Engine floors per core: scalar exp 144 x ~1.08us = ~156us is the
   pacer; PE ~147us; DVE ~150us. The kernel is a race to start the exp
   stream early (~14us) and never stall it.
 - Input DMAs split across the three DMA queues (sync/SP HWDGE,
   scalar/Act HWDGE, gpsimd SWDGE), ~64-72GB/s each, parallel. wk/wq
   lead sync/scalar so the first k/q projections start ~10us; xT
   chunks are halved across rings against their scores deadlines.
 - onehot(row) [64, S] generated on device from the identity via a
   stride-0 broadcast copy (DVE), saving 786KB of ramp DMA.
 - First four groups interleave heads so early scores need only
   k-chunk 0 (+2 groups of slack on later xT deadlines).
 - Scalar engine does exp ONLY between first and last exp. GpSimd does
   NO tensor work mid-loop (a v2 experiment showed gpsimd tensor_mul
   contends for SBUF and slows concurrent DVE ops ~50%): it only
   issues SWDGE DMAs. All multiplies/evacuations/casts on DVE.
 - vv ones+values combined: [v_h0 64 | one | v_h1 64 | one] per
   130-col block: each v-transpose needs ONE (2-block-AP) copy and
   both heads' AV lhsT slices are partition-aligned [v|one].
"""

import numpy as np

EMBED = 512
NH = 8
HD = 64
GH, GW = 64, 48
B = 2
S = GH * GW  # 3072
N_CORES = 8
NQ = S // 512  # 6 q chunks of 512
NM = S // 128  # 24 k chunks of 128
NG = NM // 2   # 12 groups of 2 k-chunks per (n, h)
KC = 4         # 512 = 4 contraction chunks of 128

_CACHE = {}


def _build_program():
    import concourse.bass as bass
    import concourse.tile as tile
    import concourse.mybir as mybir
    from concourse import bacc
    from concourse.bass import ts, ds
    from concourse.masks import make_identity

    f32 = mybir.dt.float32
    f16 = mybir.dt.float16
    EXP = mybir.ActivationFunctionType.Exp

    nc = bacc.Bacc("TRN2", target_bir_lowering=False, debug=False,
                   num_devices=N_CORES)

    def inp(name, shape):
        return nc.dram_tensor(name, shape, f16, kind="ExternalInput").ap()

    # host-prepacked layouts (see _prep_core_inputs)
    xT_d = inp("xT", [128, NQ * 2048])        # [p, n*2048 + c*512 + col]
    wqkv_d = inp("wqkv", [128, 3 * 512])      # [p, (q|k|v)*512 + c*128 + col]
    rowr_d = [inp(f"rowr{h}", [64, S]) for h in range(2)]      # rowr_h only
    ecol_d = [inp(f"ecol{h}", [128, 6144]) for h in range(2)]  # blocks 0,1,2,0
    wout_d = inp("wout", [HD, 2 * EMBED])
    outa_d = nc.dram_tensor("outa", [S, EMBED], f16, kind="ExternalOutput").ap()
    outb_d = nc.dram_tensor("outb", [S, EMBED], f16, kind="ExternalOutput").ap()
    den_d = nc.dram_tensor("den", [2, S], f16, kind="ExternalOutput").ap()

    with tile.TileContext(nc) as tc:
        with (
            tc.tile_pool(name="const", bufs=1) as cpool,
            tc.tile_pool(name="vtwp", bufs=3) as vtwp,
            tc.tile_pool(name="ptp", bufs=6) as ptp,
            tc.tile_pool(name="ptmp", bufs=8) as ptmp,
            tc.tile_pool(name="osb", bufs=3) as opool,
            tc.tile_pool(name="ps", bufs=2, space="PSUM") as ps,
        ):
            # ---- resident SBUF tensors ----
            xT = [cpool.tile([128, 2048], f16, tag=f"xT{n}", name=f"xT{n}")
                  for n in range(NQ)]
            wqkv = cpool.tile([128, 3 * 512], f16)
            wout = cpool.tile([HD, 2 * EMBED], f16)
            augLR = [cpool.tile([128, 2 * S], f16, tag=f"augLR{h}",
                                name=f"augLR{h}") for h in range(2)]
            ecol = [cpool.tile([128, 6144], f16, tag=f"ecol{h}",
                               name=f"ecol{h}") for h in range(2)]
            # per 130-block: [v_h0 64 | one | v_h1 64 | one] so each
            # head's AV lhsT slice is a partition-0-aligned [v|one]
            vv = cpool.tile([128, NM * 130], f16)
            outT = [cpool.tile([65, S], f16, tag=f"outT{h}", name=f"outT{h}")
                    for h in range(2)]
            ident = cpool.tile([128, 128], f16)

            # ---- input DMA rings ------------------------------------
            # HWDGE ring depth is 4: the 5th+ dma_start on a queue
            # BLOCKS the queue until a transfer slot frees. The scalar
            # queue therefore gets EXACTLY 4 pre-exp items (its queue
            # must be free when exp(0) is ready ~16us); sync and gpsimd
            # queues have nothing time-critical and absorb the rest.
            nc.gpsimd.dma_start(out=wqkv[:, 1024:1536],
                                in_=wqkv_d[:, 1024:1536])    # wv
            make_identity(nc, ident[:, :])
            nc.gpsimd.memset(vv[:, 0:512], 1.0)
            nc.gpsimd.memset(vv[:, 512:NM * 130], 1.0)
            nc.gpsimd.dma_start(out=ecol[1][:, 0:1024],
                                in_=ecol_d[1][:, 0:1024])
            nc.gpsimd.dma_start(out=ecol[1][:, 1024:2048],
                                in_=ecol_d[1][:, 1024:2048])
            nc.gpsimd.dma_start(out=xT[1][:, 1024:2048],
                                in_=xT_d[:, ds(1 * 2048 + 1024, 1024)])
            nc.gpsimd.dma_start(out=xT[2][:, 0:1024],
                                in_=xT_d[:, ds(2 * 2048, 1024)])
            nc.gpsimd.dma_start(out=xT[2][:, 1024:2048],
                                in_=xT_d[:, ds(2 * 2048 + 1024, 1024)])
            nc.gpsimd.dma_start(out=xT[3][:, 1024:2048],
                                in_=xT_d[:, ds(3 * 2048 + 1024, 1024)])
            nc.gpsimd.dma_start(out=xT[4][:, 0:1024],
                                in_=xT_d[:, ds(4 * 2048, 1024)])
            nc.gpsimd.dma_start(out=xT[5][:, 1024:2048],
                                in_=xT_d[:, ds(5 * 2048 + 1024, 1024)])

            # sync (SP HWDGE): wk first (gates kproj0), xT0 c0/c2,
            # rowr n0 slices, xT1a, ecol0 panel-0, late a-halves
            nc.sync.dma_start(out=wqkv[:, 512:1024], in_=wqkv_d[:, 512:1024])
            nc.sync.dma_start(out=xT[0][:, 0:512], in_=xT_d[:, ds(0, 512)])
            nc.sync.dma_start(out=xT[0][:, 1024:1536],
                              in_=xT_d[:, ds(1024, 512)])
            nc.sync.dma_start(out=augLR[0][64:128, ds(S, 512)],
                              in_=rowr_d[0][:, 0:512])
            nc.sync.dma_start(out=augLR[1][64:128, ds(S, 512)],
                              in_=rowr_d[1][:, 0:512])
            nc.sync.dma_start(out=xT[1][:, 0:1024],
                              in_=xT_d[:, ds(1 * 2048, 1024)])
            nc.sync.dma_start(out=ecol[0][:, 0:1024],
                              in_=ecol_d[0][:, 0:1024])
            nc.sync.dma_start(out=xT[3][:, 0:1024],
                              in_=xT_d[:, ds(3 * 2048, 1024)])
            nc.sync.dma_start(out=xT[5][:, 0:1024],
                              in_=xT_d[:, ds(5 * 2048, 1024)])

            # scalar (Act HWDGE): 5 items — the 5th issue ring-blocks
            # the queue only until the 1st transfer completes (~12us),
            # still well before exp(0)
            nc.scalar.dma_start(out=wqkv[:, 0:512], in_=wqkv_d[:, 0:512])
            nc.scalar.dma_start(out=xT[0][:, 512:1024],
                                in_=xT_d[:, ds(512, 512)])
            nc.scalar.dma_start(out=xT[0][:, 1536:2048],
                                in_=xT_d[:, ds(1536, 512)])
            nc.scalar.dma_start(out=xT[4][:, 1024:2048],
                                in_=xT_d[:, ds(4 * 2048 + 1024, 1024)])
            nc.scalar.dma_start(out=ecol[0][:, 1024:2048],
                                in_=ecol_d[0][:, 1024:2048])

            # ---- onehot(row) [64, S] generated on device ------------
            # ohr = kron(I64, ones(1,48)): identity columns repeated 48x
            # via a stride-0 inner AP; split at col 1056 (=48*22)
            def emit_ohr(h, c0, c1, eng):
                src = ident[0:64, c0:c1]
                src = bass.AP(src.tensor, src.offset, src.ap + [[0, 48]])
                dst = augLR[h][64:128, 48 * c0:48 * c1]
                dst = bass.AP(dst.tensor, dst.offset,
                              [dst.ap[0], [48, c1 - c0], [1, 48]])
                if hasattr(eng, "tensor_copy"):
                    eng.tensor_copy(dst, src)
                else:
                    eng.copy(dst, src)

            # ---- group order: heads fully interleaved --------------
            # (n,0,g),(n,1,g) pairs: each k-chunk pair is consumed over
            # TWO groups, halving the k-side DMA bandwidth demand during
            # the ramp (xT_n deadline ~G(4n) instead of ~G(2n)).
            groups = [(n, h, g) for n in range(NQ) for g in range(NG)
                      for h in (0, 1)]
            NGRP = len(groups)
            assert NGRP == NQ * 2 * NG

            live = {}
            acc = {}

            def emit_scores(i):
                n, h, g = groups[i]
                st = ps.tile([128, 1024], f32, tag="st", name="st")
                for k in range(2):
                    m = 2 * g + k
                    nc.tensor.matmul(st[:, ts(k, 512)],
                                     augLR[h][:, ts(m, 128)],
                                     augLR[h][:, ds(S + n * 512, 512)],
                                     start=True, stop=True)
                live[("st", i)] = st

            def emit_expmul(i):
                n, h, g = groups[i]
                st = live.pop(("st", i))
                pt = ptp.tile([128, 1024], f16, tag="pt", name="pt")
                nc.scalar.activation(pt[:, :], st[:, :], EXP)
                ptm = ptmp.tile([128, 1024], f16, tag="ptm", name="ptm")
                esl = ecol[h][:, ds((n % 3) * 2048 + (2 * g % 3) * 512,
                                    1024)]
                # NOTE: scalar_tensor_tensor looks 4x-capable in the
                # cost model but measures ~2.5x SLOWER than tensor_mul
                # on hardware; plain tensor_tensor 2x (~640ns) is best.
                nc.vector.tensor_mul(ptm[:, :], pt[:, :], esl)
                live[("ptm", i)] = ptm

            def emit_av(i):
                n, h, g = groups[i]
                ptm = live.pop(("ptm", i))
                if g == 0:
                    acc[(n, h)] = ps.tile([65, 512], f32, tag="acc",
                                          name="acc")
                a = acc[(n, h)]
                for k in range(2):
                    m = 2 * g + k
                    nc.tensor.matmul(a[:, :], vv[:, ds(m * 130 + 65 * h, 65)],
                                     ptm[:, ts(k, 512)],
                                     start=(m == 0), stop=(m == NM - 1))
                if g == NG - 1:
                    # acc rows = [v64, den]: emit the unnormalized head
                    # output + den row; host divides and combines
                    # (device-side normalization was tried: its DVE
                    # recip/broadcast chain costs more than it saves).
                    # Last chunk: split halves so the final tail units
                    # pipeline with the copy.
                    if n == NQ - 1:
                        nc.vector.tensor_copy(
                            outT[h][:, ds(n * 512, 256)], a[:, 0:256])
                        nc.vector.tensor_copy(
                            outT[h][:, ds(n * 512 + 256, 256)], a[:, 256:512])
                    else:
                        nc.vector.tensor_copy(outT[h][:, ts(n, 512)],
                                              a[:, :])
                    del acc[(n, h)]

            def emit_tail_unit(t, h, tag="fp", cast_scalar=False,
                               ring="default"):
                fp = ps.tile([128, 512], f32, tag=tag, name="fp")
                nc.tensor.matmul(fp[:, :], outT[h][0:64, ts(t, 128)],
                                 wout[:, ds(h * EMBED, EMBED)],
                                 start=True, stop=True)
                osb = opool.tile([128, 512], f16, tag="osb", name="osb")
                if cast_scalar:
                    nc.scalar.copy(osb[:, :], fp[:, :])
                else:
                    nc.vector.tensor_copy(osb[:, :], fp[:, :])
                if ring == "default":
                    eng = nc.sync if h == 0 else nc.gpsimd
                else:
                    eng = ring
                out_d = outa_d if h == 0 else outb_d
                eng.dma_start(out=out_d[ts(t, 128), :], in_=osb[:, :])

            def proj(dst_tag, w_ofs, n, tag):
                p = ps.tile([128, 512], f32, tag=tag, name=f"p{dst_tag}")
                for c in range(KC):
                    nc.tensor.matmul(p[:, :], wqkv[:, ds(w_ofs + c * 128, 128)],
                                     xT[n][:, ts(c, 512)],
                                     start=(c == 0), stop=(c == KC - 1))
                return p

            def emit_kevac(n, pk, h_first_only=False):
                nc.vector.tensor_copy(augLR[0][0:64, ts(n, 512)], pk[0:64, :])
                if not h_first_only:
                    nc.vector.tensor_copy(augLR[1][0:64, ts(n, 512)],
                                          pk[64:128, :])

            def emit_qproj_copy(pq, n, h_list=(0, 1)):
                for h in h_list:
                    nc.vector.tensor_copy(augLR[h][0:64, ds(S + n * 512, 512)],
                                          pq[64 * h:64 * h + 64, :])

            def _vtr(n, mm, vtw):
                m = n * 4 + mm
                ptr = ps.tile([128, 128], f16, tag="fp", name="ptr")
                nc.tensor.transpose(ptr[:, :], vtw[:, ts(mm, 128)],
                                    ident[:, :])
                # one copy into both heads' v slots, skipping the
                # ones column at block offset 64 via a 2-block AP
                s = ptr[:, :]
                s = bass.AP(s.tensor, s.offset,
                            [s.ap[0], [64, 2], [1, 64]])
                d = vv[:, ds(m * 130, 129)]
                d = bass.AP(d.tensor, d.offset,
                            [d.ap[0], [65, 2], [1, 64]])
                nc.vector.tensor_copy(d, s)

            # split across two slots: pv + 4 transposes put 5 tiles
            # through the 2-buffer fp ring in one slot, exhausting the
            # PE 4-deep wait queue and delaying the next scores
            vstash = {}

            def emit_vproj_a(n):
                pv = proj("v", 1024, n, "fp")
                vtw = vtwp.tile([128, 512], f16, tag="vtw", name="vtw")
                nc.vector.tensor_copy(vtw[:, :], pv[:, :])
                vstash[n] = vtw
                for mm in range(2):
                    _vtr(n, mm, vtw)

            def emit_vproj_b(n):
                vtw = vstash.pop(n)
                for mm in range(2, 4):
                    _vtr(n, mm, vtw)

            # ---- ramp ----------------------------------------------
            warm = ps.tile([128, 512], f32, tag="st", name="warm")

            def emit_warm(k):
                for _ in range(k):
                    nc.tensor.matmul(warm[:, :], ident[:, :], vv[:, 0:512],
                                     start=True, stop=True)

            emit_ohr(0, 0, 22, nc.vector)
            emit_ohr(1, 0, 22, nc.vector)
            # cols 1056:3072 on the scalar queue: it is idle until
            # exp(0) ~18us, and this frees the DVE for the early muls
            emit_ohr(0, 22, 64, nc.scalar)
            emit_ohr(1, 22, 64, nc.scalar)

            emit_warm(6)
            pk0 = proj("k", 512, 0, "fp")
            emit_warm(2)
            pq0 = proj("q", 0, 0, "fp")
            # h0 evacs first: scores(0) only needs head 0
            emit_kevac(0, pk0, h_first_only=True)
            emit_qproj_copy(pq0, 0, h_list=(0,))
            emit_scores(0)   # (0,0,0)
            nc.vector.tensor_copy(augLR[1][0:64, ts(0, 512)], pk0[64:128, :])
            emit_qproj_copy(pq0, 0, h_list=(1,))
            emit_expmul(0)
            emit_vproj_a(0)  # vv chunks 0..3; AV(G0) lands at loop i=3
            emit_vproj_b(0)
            emit_warm(3)
            emit_scores(1)   # (0,1,0)
            emit_expmul(1)
            pk1 = proj("k", 512, 1, "fp")
            emit_kevac(1, pk1)

            # ---- main loop (AV lagged 3 groups behind scores/exp) ----
            pq = {}
            for i in range(2, NGRP):
                n, h, g = groups[i]
                emit_scores(i)
                emit_expmul(i)
                if i >= 3:
                    emit_av(i - 3)
                # kproj/vproj alternate slots (together in one slot
                # they exhaust the PE 4-deep wait queue and head-block
                # the next scores)
                if i in (4, 8, 12, 16, 20):
                    emit_vproj_a(i // 4)
                if i in (5, 9, 13, 17, 21):
                    emit_vproj_b(i // 4)
                # kproj nk consumed by scores (0,*,2(nk-1)) = G(4nk-4)
                if i in (2, 6, 10, 14):
                    nk = (i + 6) // 4
                    pkn = proj("k", 512, nk, "fp")
                    emit_kevac(nk, pkn)
                # lazy-constant DMA issues from gpsimd queue slots
                if i == 3:   # rowr0 n1..n5
                    nc.gpsimd.dma_start(out=augLR[0][64:128, ds(S + 512, 2560)],
                                        in_=rowr_d[0][:, ds(512, 2560)])
                if i == 5:   # rowr1 n1..n5
                    nc.gpsimd.dma_start(out=augLR[1][64:128, ds(S + 512, 2560)],
                                        in_=rowr_d[1][:, ds(512, 2560)])
                if i == 7:   # wout (first tail unit ~G26)
                    nc.gpsimd.dma_start(out=wout[:, :], in_=wout_d[:, :])
                # ecol p1/p2 panels ride the sync ring (idle after its
                # ramp items; the gpsimd ring is backed up with xT)
                if i == 9:   # ecol0 panel 1 ((1,0) muls ~G24)
                    nc.sync.dma_start(out=ecol[0][:, 2048:4096],
                                      in_=ecol_d[0][:, 2048:4096])
                if i == 15:  # ecol1 panel 1
                    nc.sync.dma_start(out=ecol[1][:, 2048:4096],
                                      in_=ecol_d[1][:, 2048:4096])
                if i == 21:  # ecol0 panel 2
                    nc.sync.dma_start(out=ecol[0][:, 4096:6144],
                                      in_=ecol_d[0][:, 4096:6144])
                if i == 27:  # ecol1 panel 2
                    nc.sync.dma_start(out=ecol[1][:, 4096:6144],
                                      in_=ecol_d[1][:, 4096:6144])
                # q-proj n=1 during the (0,1) block
                if (n, h, g) == (0, 1, 3):
                    pq[1] = proj("q", 0, 1, "fp")
                if (n, h, g) == (0, 1, 5):
                    emit_qproj_copy(pq.pop(1), 1)
                # output-projection tail units: outT[*] chunk n-1 is
                # complete by loop i=24n+1; slots g=4..7 of both heads
                # in block n emit its 8 units (g>=4 keeps the unit
                # casts clear of the block-boundary DVE burst of outT
                # copies, which was stalling the muls ~1.7us per block)
                if n >= 1 and 4 <= g < 8:
                    emit_tail_unit(4 * (n - 1) + (g - 4), h)
                if h == 0 and g == 6 and 2 <= n + 2 < NQ:
                    pq[n + 2] = proj("q", 0, n + 2, "fp")
                if h == 0 and g == 8 and 2 <= n + 2 < NQ:
                    emit_qproj_copy(pq.pop(n + 2), n + 2)
            # drain: h0's final units start right after (5,0,11)'s AV,
            # overlapping the last group's AV and spreading the output
            # DMAs ~3us earlier. All exps are done by now, so half the
            # casts go to the scalar engine.
            emit_av(NGRP - 3)   # (5,1,10)
            emit_av(NGRP - 2)   # (5,0,11) -> outT[0] chunk 5
            nc.sync.dma_start(out=den_d[0:1, :], in_=outT[0][64:65, :])
            h0_rings = [nc.sync, nc.gpsimd, nc.sync, nc.gpsimd]
            for tt in range(4):
                emit_tail_unit(4 * (NQ - 1) + tt, 0,
                               tag="fp" if tt % 2 == 0 else "st",
                               cast_scalar=(tt % 2 == 1),
                               ring=h0_rings[tt])
            emit_av(NGRP - 1)   # (5,1,11) -> outT[1] chunk 5
            nc.scalar.dma_start(out=den_d[1:2, :], in_=outT[1][64:65, :])
            h1_rings = [nc.scalar, nc.sync, nc.gpsimd, nc.scalar]
            for tt in range(4):
                emit_tail_unit(4 * (NQ - 1) + tt, 1,
                               tag="st" if tt % 2 == 0 else "fp",
                               cast_scalar=(tt % 2 == 1),
                               ring=h1_rings[tt])

    nc.compile()
    return nc


def _get_nc():
    if "nc" not in _CACHE:
        _CACHE["nc"] = _build_program()
    return _CACHE["nc"]


def _prep_core_inputs(x, w_qkv, w_out, rel_row_tab, rel_col_tab):
    """Per-core input dicts (host-side shard + constant precompute)."""
    bf = np.float16
    x = np.asarray(x, np.float32)
    w_qkv = np.asarray(w_qkv, np.float32)
    w_out = np.asarray(w_out, np.float32)
    rel_row_tab = np.asarray(rel_row_tab, np.float32)
    rel_col_tab = np.asarray(rel_col_tab, np.float32)

    ri = np.arange(S) // GW
    row_idx = ri[None, :] - np.arange(64)[:, None] + 63   # [64, S]

    jj = np.arange(128)
    ii = np.arange(512)
    ecol_idx = np.zeros((3, 4, 128, 512), np.int64)
    for q3 in range(3):
        for bp in range(4):
            cj = (32 * (bp % 3) + jj) % 48
            c_i = (32 * q3 + ii) % 48
            ecol_idx[q3, bp] = c_i[None, :] - cj[:, None] + 47
    ecol_idx = ecol_idx.transpose(2, 0, 1, 3).reshape(128, 6144)

    scale = HD ** -0.5
    in_maps = []
    for c in range(N_CORES):
        b = c // 4
        h0 = 2 * (c % 4)
        h1 = h0 + 1
        xT = np.ascontiguousarray(x[b].reshape(S, EMBED).T)   # [E, S]
        xTn = xT.reshape(KC, 128, NQ, 512).transpose(1, 2, 0, 3)
        def wslice(base, h):
            return w_qkv[:, base + h * HD: base + (h + 1) * HD]
        def pack(base, mul=1.0):
            w = np.concatenate([wslice(base, h0), wslice(base, h1)],
                               axis=1) * mul                  # [512, 128]
            return w.reshape(KC, 128, 128).transpose(1, 0, 2).reshape(128, 512)
        wqkv = np.concatenate([pack(0, scale), pack(EMBED), pack(2 * EMBED)],
                              axis=1)

        in_maps.append({
            "xT": np.ascontiguousarray(xTn.reshape(128, NQ * 2048)).astype(bf),
            "wqkv": np.ascontiguousarray(wqkv).astype(bf),
            "rowr0": np.ascontiguousarray(
                rel_row_tab[row_idx, h0]).astype(bf),
            "rowr1": np.ascontiguousarray(
                rel_row_tab[row_idx, h1]).astype(bf),
            "ecol0": np.exp(rel_col_tab[ecol_idx, h0]).astype(bf),
            "ecol1": np.exp(rel_col_tab[ecol_idx, h1]).astype(bf),
            "wout": np.concatenate(
                [w_out[h0 * HD:(h0 + 1) * HD, :],
                 w_out[h1 * HD:(h1 + 1) * HD, :]], axis=1).astype(bf),
        })
    return in_maps


def _run(inputs, trace=False):
    from concourse.bass_utils import run_bass_kernel_spmd
    nc = _get_nc()
    in_maps = _prep_core_inputs(**inputs)
    res = run_bass_kernel_spmd(nc, in_maps, list(range(N_CORES)), trace=trace)
    acc = np.zeros((B, S, EMBED), np.float32)
    for c in range(N_CORES):
        r = res.results[c]
        den = np.asarray(r["den"], np.float32)          # [2, S]
        acc[c // 4] += np.asarray(r["outa"], np.float32) / den[0][:, None]
        acc[c // 4] += np.asarray(r["outb"], np.float32) / den[1][:, None]
    return acc.reshape(B, GH, GW, EMBED), res


def kernel(x, w_qkv, w_out, rel_row_tab, rel_col_tab):
    out, _ = _run(dict(x=x, w_qkv=w_qkv, w_out=w_out,
                       rel_row_tab=rel_row_tab, rel_col_tab=rel_col_tab))
    return out
